# revision 2
# baseline (speedup 1.0000x reference)
"""Trainium2 Bass kernel for nn_Block_47261820125190 (dense transformer block).

Contract: kernel(**inputs) takes FULL inputs (x [8192,16,256] + weights),
shards batch across 8 NeuronCores (data parallel), runs a fused Bass/Tile
kernel per core, returns FULL output [8192,16,256] fp32.
"""

import sys

for p in ("/opt/trn_rl_repo",):
    if p not in sys.path:
        sys.path.insert(0, p)

import numpy as np
import ml_dtypes
import os as _os
SKIP_ATTN = _os.environ.get('SKIP_ATTN','0')=='1'
SKIP_MLP = _os.environ.get('SKIP_MLP','0')=='1'
SKIP_SMAX = _os.environ.get('SKIP_SMAX','0')=='1'
from contextlib import ExitStack

import concourse.bass as bass
import concourse.tile as tile
from concourse import bacc
from concourse import mybir
from concourse.bass_utils import run_bass_kernel_spmd

# Model dims (hardcoded per spec)
B, T, C, H = 8192, 16, 256, 4
HS = C // H          # 64
FF = 4 * C           # 1024
EPS = 1e-5
NCORES = 8
NTOK = (B // NCORES) * T   # 16384 tokens per core
TT = 512                   # tokens per big tile
NST = TT // 128            # 4 subtiles of 128 tokens
NTILES = NTOK // TT        # 32
CINV = float(C) ** -0.5

F32 = mybir.dt.float32
F32R = mybir.dt.float32r
BF16 = mybir.dt.bfloat16


def _r(ap):
    return ap.bitcast(F32R)


def build_kernel():
    nc = bacc.Bacc(None)

    x_d = nc.declare_dram_parameter("x", [NTOK, C], F32, isOutput=False)
    wqkv_d = nc.declare_dram_parameter("wqkv", [C, 3 * C], F32R, isOutput=False)
    bqk_d = nc.declare_dram_parameter("bqk", [128, 4], F32, isOutput=False)
    bv_d = nc.declare_dram_parameter("bv", [C], F32, isOutput=False)
    wo_d = nc.declare_dram_parameter("wo", [C, C], F32R, isOutput=False)
    bo_d = nc.declare_dram_parameter("bo", [C], F32, isOutput=False)
    w1_d = nc.declare_dram_parameter("w1", [C, FF], F32R, isOutput=False)
    b1m_d = nc.declare_dram_parameter("b1m", [128, FF // 128], F32, isOutput=False)
    w2_d = nc.declare_dram_parameter("w2", [FF, C], F32R, isOutput=False)
    b2_d = nc.declare_dram_parameter("b2", [C], F32, isOutput=False)
    mask_d = nc.declare_dram_parameter("maskb", [128, 128], F32, isOutput=False)
    id_d = nc.declare_dram_parameter("ident", [128, 128], F32R, isOutput=False)
    idb_d = nc.declare_dram_parameter("identb", [128, 128], BF16, isOutput=False)
    out_d = nc.declare_dram_parameter("out", [NTOK, C], F32, isOutput=True)

    def bcast(ap_1d, n):
        # view a [n] dram vector as [128, n] with 0-step partition dim
        return bass.AP(tensor=ap_1d.tensor, offset=ap_1d.offset,
                       ap=[[0, 128]] + list(ap_1d.ap))

    with TileCtx(nc) as tc:
        ctx = tc._ctx
        singles = ctx.enter_context(tc.tile_pool(name="singles", bufs=1))
        acts = ctx.enter_context(tc.tile_pool(name="acts", bufs=2))
        small = ctx.enter_context(tc.tile_pool(name="small", bufs=8))
        attnp = ctx.enter_context(tc.tile_pool(name="attnp", bufs=3))
        psA = ctx.enter_context(tc.tile_pool(name="psA", bufs=3, space="PSUM"))
        psB = ctx.enter_context(tc.tile_pool(name="psB", bufs=2, space="PSUM"))
        psC = ctx.enter_context(tc.tile_pool(name="psC", bufs=3, space="PSUM"))

        # ---- persistent weights/constants in SBUF ----
        wqkv_sb = []
        for k in range(2):
            t_ = singles.tile([128, 3 * C], F32R, tag=f"wqkv{k}")
            nc.sync.dma_start(out=t_, in_=wqkv_d[k * 128:(k + 1) * 128, :])
            wqkv_sb.append(t_)
        wo_sb = []
        for k in range(2):
            t_ = singles.tile([128, C], F32R, tag=f"wo{k}")
            nc.sync.dma_start(out=t_, in_=wo_d[k * 128:(k + 1) * 128, :])
            wo_sb.append(t_)
        w1_sb = []
        for k in range(2):
            t_ = singles.tile([128, FF], F32R, tag=f"w1{k}")
            nc.sync.dma_start(out=t_, in_=w1_d[k * 128:(k + 1) * 128, :])
            w1_sb.append(t_)
        w2_sb = []
        for k in range(8):
            t_ = singles.tile([128, C], F32R, tag=f"w2{k}")
            nc.sync.dma_start(out=t_, in_=w2_d[k * 128:(k + 1) * 128, :])
            w2_sb.append(t_)
        bqk_sb = singles.tile([128, 4], F32, tag="bqk")
        nc.sync.dma_start(out=bqk_sb, in_=bqk_d[:, :])
        b1m_sb = singles.tile([128, FF // 128], F32, tag="b1m")
        nc.sync.dma_start(out=b1m_sb, in_=b1m_d[:, :])
        bv_b = singles.tile([128, C], F32, tag="bv")
        nc.sync.dma_start(out=bv_b, in_=bcast(bv_d[:], C))
        bo_b = singles.tile([128, C], F32, tag="bo")
        nc.sync.dma_start(out=bo_b, in_=bcast(bo_d[:], C))
        b2_b = singles.tile([128, C], F32, tag="b2")
        nc.sync.dma_start(out=b2_b, in_=bcast(b2_d[:], C))
        mask_sb = singles.tile([128, 128], F32, tag="mask")
        nc.sync.dma_start(out=mask_sb, in_=mask_d[:, :])
        idf_sb = singles.tile([128, 128], F32R, tag="idf")
        nc.sync.dma_start(out=idf_sb, in_=id_d[:, :])
        idb_sb = singles.tile([128, 128], BF16, tag="idb")
        nc.sync.dma_start(out=idb_sb, in_=idb_d[:, :])
        eps_sb = singles.tile([128, 1], F32, tag="eps")
        nc.vector.memset(eps_sb, EPS)

        def layernorm(x_sb, h_sb, st):
            stats = small.tile([128, 6], F32, tag="stats")
            mv = small.tile([128, 2], F32, tag="mv")
            rstd = small.tile([128, 1], F32, tag="rstd")
            nc.vector.bn_stats(out=stats, in_=x_sb)
            nc.vector.bn_aggr(out=mv, in_=stats)
            nc.scalar.activation(out=rstd, in_=mv[:, 1:2],
                                 func=mybir.ActivationFunctionType.Sqrt,
                                 bias=eps_sb, scale=1.0)
            nc.vector.reciprocal(out=rstd, in_=rstd)
            nc.vector.tensor_scalar(out=h_sb, in0=x_sb,
                                    scalar1=mv[:, 0:1], scalar2=rstd,
                                    op0=mybir.AluOpType.subtract,
                                    op1=mybir.AluOpType.mult)

        for it in range(NTILES):
            base = (it % (NTOK // TT)) * TT
            # ---- load x, LN1, transpose h ----
            x_sb = acts.tile([128, NST, C], F32, tag="x")
            h_sb = acts.tile([128, NST, C], F32R, tag="h")
            hT_sb = [acts.tile([128, TT], F32R, tag=f"hT{k}", name=f"hT{k}") for k in range(2)]
            for st in range(NST):
                nc.sync.dma_start(
                    out=x_sb[:, st, :],
                    in_=x_d[base + st * 128: base + (st + 1) * 128, :])
                layernorm(x_sb[:, st, :], h_sb[:, st, :], st)
                for cc in range(2):
                    tp = psB.tile([128, 128], F32, tag="tp")
                    nc.tensor.transpose(out=_r(tp), in_=_r(h_sb[:, st, cc * 128:(cc + 1) * 128]),
                                        identity=_r(idf_sb))
                    nc.vector.tensor_copy(out=hT_sb[cc][:, st * 128:(st + 1) * 128], in_=tp)

            # ---- QKV ----
            qT_sb = [attnp.tile([128, TT], BF16, tag=f"qT{m}", name=f"qT{m}") for m in range(2)]
            kT_sb = [attnp.tile([128, TT], BF16, tag=f"kT{m}", name=f"kT{m}") for m in range(2)]
            for m in range(4):  # 0,1 -> q chunks; 2,3 -> k chunks
                ps = psA.tile([128, TT], F32, tag="psA")
                for k in range(2):
                    nc.tensor.matmul(out=ps,
                                     lhsT=_r(wqkv_sb[k][:, m * 128:(m + 1) * 128]),
                                     rhs=_r(hT_sb[k]),
                                     start=(k == 0), stop=(k == 1))
                dst = qT_sb[m] if m < 2 else kT_sb[m - 2]
                nc.vector.tensor_scalar(out=dst, in0=ps,
                                        scalar1=bqk_sb[:, m:m + 1],
                                        scalar2=CINV if m < 2 else 1.0,
                                        op0=mybir.AluOpType.add,
                                        op1=mybir.AluOpType.mult)
            v_sb = attnp.tile([128, NST, C], BF16, tag="v")
            for st in range(NST):
                ps = psC.tile([128, C], F32, tag="psC")
                for k in range(2):
                    nc.tensor.matmul(out=ps,
                                     lhsT=_r(hT_sb[k][:, st * 128:(st + 1) * 128]),
                                     rhs=_r(wqkv_sb[k][:, 2 * C:3 * C]),
                                     start=(k == 0), stop=(k == 1))
                nc.vector.tensor_add(out=v_sb[:, st, :], in0=ps, in1=bv_b)

            # ---- attention ----
            attnT_sb = [acts.tile([128, TT], F32R, tag=f"aT{hc}", name=f"aT{hc}") for hc in range(2)]
            attnT_ps = [psA.tile([128, TT], F32, tag="psA", name=f"aTps{hc}") for hc in range(2)]
            if SKIP_ATTN:
                for hc in range(2):
                    nc.vector.tensor_copy(out=attnT_sb[hc], in_=hT_sb[hc])
            for st in range(NST if not SKIP_ATTN else 0):
                for h in range(H):
                    hc, off = h // 2, 64 * (h % 2)
                    sl = slice(st * 128, (st + 1) * 128)
                    s_ps = psC.tile([128, 128], F32, tag="psC")
                    nc.tensor.matmul(out=s_ps,
                                     lhsT=qT_sb[hc][off:off + 64, sl],
                                     rhs=kT_sb[hc][off:off + 64, sl],
                                     start=True, stop=True)
                    wn_sb = small.tile([128, 128], BF16, tag="wn")
                    if SKIP_SMAX:
                        nc.vector.tensor_copy(out=wn_sb, in_=s_ps)
                    else:
                        nc.vector.tensor_add(out=s_ps, in0=s_ps, in1=mask_sb)
                        w_sb = small.tile([128, 128], BF16, tag="w")
                        rsum = small.tile([128, 1], F32, tag="rsum")
                        nc.scalar.activation(out=w_sb, in_=s_ps,
                                             func=mybir.ActivationFunctionType.Exp,
                                             accum_out=rsum)
                        rcp = small.tile([128, 1], F32, tag="rcp")
                        nc.vector.reciprocal(out=rcp, in_=rsum)
                        nc.gpsimd.tensor_scalar_mul(out=wn_sb, in0=w_sb, scalar1=rcp)
                    wt_ps = psB.tile([128, 128], BF16, tag="tp")
                    nc.tensor.transpose(out=wt_ps, in_=wn_sb, identity=idb_sb)
                    wt_sb = small.tile([128, 128], BF16, tag="wt")
                    nc.vector.tensor_copy(out=wt_sb, in_=wt_ps)
                    nc.tensor.matmul(out=attnT_ps[hc][off:off + 64, sl],
                                     lhsT=v_sb[:, st, h * 64:(h + 1) * 64],
                                     rhs=wt_sb,
                                     start=True, stop=True)
            for hc in range(2 if not SKIP_ATTN else 0):
                nc.vector.tensor_copy(out=attnT_sb[hc], in_=attnT_ps[hc])

            # ---- Wo + residual, LN2, transpose h2 ----
            x2_sb = acts.tile([128, NST, C], F32, tag="x2")
            h2_sb = acts.tile([128, NST, C], F32R, tag="h2")
            h2T_sb = [acts.tile([128, TT], F32R, tag=f"h2T{k}", name=f"h2T{k}") for k in range(2)]
            for st in range(NST):
                ps = psC.tile([128, C], F32, tag="psC")
                for hc in range(2):
                    nc.tensor.matmul(out=ps,
                                     lhsT=_r(attnT_sb[hc][:, st * 128:(st + 1) * 128]),
                                     rhs=_r(wo_sb[hc]),
                                     start=(hc == 0), stop=(hc == 1))
                nc.vector.scalar_tensor_tensor(out=x2_sb[:, st, :], in0=ps,
                                               scalar=1.0, in1=x_sb[:, st, :],
                                               op0=mybir.AluOpType.mult,
                                               op1=mybir.AluOpType.add)
                nc.gpsimd.tensor_add(out=x2_sb[:, st, :], in0=x2_sb[:, st, :], in1=bo_b)
                layernorm(x2_sb[:, st, :], h2_sb[:, st, :], st)
                for cc in range(2):
                    tp = psB.tile([128, 128], F32, tag="tp")
                    nc.tensor.transpose(out=_r(tp), in_=_r(h2_sb[:, st, cc * 128:(cc + 1) * 128]),
                                        identity=_r(idf_sb))
                    nc.vector.tensor_copy(out=h2T_sb[cc][:, st * 128:(st + 1) * 128], in_=tp)

            # ---- MLP ----
            m1_sb = [acts.tile([128, TT], F32R, tag=f"m1{mf}", name=f"m1{mf}") for mf in range(8)]
            for mf in range(8 if not SKIP_MLP else 0):
                ps = psA.tile([128, TT], F32, tag="psA")
                for k in range(2):
                    nc.tensor.matmul(out=ps,
                                     lhsT=_r(w1_sb[k][:, mf * 128:(mf + 1) * 128]),
                                     rhs=_r(h2T_sb[k]),
                                     start=(k == 0), stop=(k == 1))
                nc.scalar.activation(out=m1_sb[mf], in_=ps,
                                     func=mybir.ActivationFunctionType.Relu,
                                     bias=b1m_sb[:, mf:mf + 1], scale=1.0)
            o_sb = acts.tile([128, NST, C], F32, tag="o")
            for st in range(NST):
                ps = psC.tile([128, C], F32, tag="psC")
                if SKIP_MLP:
                    nc.vector.memset(ps, 0.0)
                for mf in range(8 if not SKIP_MLP else 0):
                    nc.tensor.matmul(out=ps,
                                     lhsT=_r(m1_sb[mf][:, st * 128:(st + 1) * 128]),
                                     rhs=_r(w2_sb[mf]),
                                     start=(mf == 0), stop=(mf == 7))
                nc.vector.scalar_tensor_tensor(out=o_sb[:, st, :], in0=ps,
                                               scalar=1.0, in1=x2_sb[:, st, :],
                                               op0=mybir.AluOpType.mult,
                                               op1=mybir.AluOpType.add)
                nc.gpsimd.tensor_add(out=o_sb[:, st, :], in0=o_sb[:, st, :], in1=b2_b)
                nc.sync.dma_start(
                    out=out_d[base + st * 128: base + (st + 1) * 128, :],
                    in_=o_sb[:, st, :])
    nc.finalize()
    return nc


class TileCtx:
    """TileContext wrapper carrying an ExitStack for pools."""

    def __init__(self, nc):
        self._tc = tile.TileContext(nc)
        self._ctx = ExitStack()

    def __enter__(self):
        tc = self._tc.__enter__()
        tc._ctx = self._ctx
        return tc

    def __exit__(self, *a):
        self._ctx.close()
        return self._tc.__exit__(*a)


def _prep_weights(Wq, Wk, Wv, Wo, bo, W1, b1, W2, b2, g1, be1, g2, be2):
    f = np.float32
    Wqf = np.transpose(Wq, (1, 0, 2)).reshape(C, C)  # [c,(h,d)]
    Wkf = np.transpose(Wk, (1, 0, 2)).reshape(C, C)
    Wvf = np.transpose(Wv, (1, 0, 2)).reshape(C, C)
    g1c = g1[:, None]
    wqkv = np.concatenate([g1c * Wqf, g1c * Wkf, g1c * Wvf], axis=1).astype(f)
    bq = be1 @ Wqf
    bk = be1 @ Wkf
    bv = (be1 @ Wvf).astype(f)
    bqk = np.stack([bq[:128], bq[128:], bk[:128], bk[128:]], axis=1).astype(f)
    w1 = (g2[:, None] * W1).astype(f)
    b1e = (b1 + be2 @ W1).astype(f)
    b1m = b1e.reshape(8, 128).T.copy()  # [128, 8]
    # block-diag causal additive mask [128,128]
    m = np.full((128, 128), -1e30, dtype=f)
    for j in range(8):
        blk = np.tril(np.zeros((T, T), dtype=f) + 0.0) * 0.0
        tri = np.triu(np.full((T, T), -1e30, dtype=f), 1)
        m[j * T:(j + 1) * T, j * T:(j + 1) * T] = tri
    return dict(wqkv=wqkv, bqk=bqk, bv=bv, wo=Wo.astype(f), bo=bo.astype(f),
                w1=w1, b1m=b1m.astype(f), w2=W2.astype(f), b2=b2.astype(f),
                maskb=m, ident=np.eye(128, dtype=f),
                identb=np.eye(128).astype(ml_dtypes.bfloat16))


_CACHE = {}


def kernel(x, Wq, Wk, Wv, Wo, bo, W1, b1, W2, b2, g1, be1, g2, be2):
    x = np.asarray(x, dtype=np.float32)
    wts = _prep_weights(np.asarray(Wq), np.asarray(Wk), np.asarray(Wv),
                        np.asarray(Wo), np.asarray(bo), np.asarray(W1),
                        np.asarray(b1), np.asarray(W2), np.asarray(b2),
                        np.asarray(g1), np.asarray(be1), np.asarray(g2),
                        np.asarray(be2))
    if "nc" not in _CACHE:
        _CACHE["nc"] = build_kernel()
    nc = _CACHE["nc"]
    xs = x.reshape(NCORES, NTOK, C)
    in_maps = [dict(x=np.ascontiguousarray(xs[i]), **wts) for i in range(NCORES)]
    kres = run_bass_kernel_spmd(
        nc, in_maps, list(range(NCORES)),
        trace=_os.environ.get("KERNEL_TRACE", "0") == "1",
        tmpdir=_os.environ.get("KERNEL_TRACE_DIR") or None)
    _CACHE["last"] = kres
    res = kres.results
    out = np.stack([res[i]["out"] for i in range(NCORES)], axis=0)
    return out.reshape(B, T, C).astype(np.float32)


if __name__ == "__main__":
    nc = build_kernel()
    print("kernel traced OK")



# revision 14
# speedup vs baseline: 1.1411x; 1.1411x over previous
"""Trainium2 Bass kernel for nn_Block_47261820125190 (dense transformer block).

Contract: kernel(**inputs) takes FULL inputs (x [8192,16,256] + weights),
shards batch across 8 NeuronCores (data parallel), runs a fused Bass/Tile
kernel per core, returns FULL output [8192,16,256] fp32.

v2 design notes (vs v1 baseline at 2.28ms/core):
- fp16 operands for all matmuls (full PE rate, avoids fp32 power throttle)
- softmax normalization folded into token-major scale before the w
  transpose; no gpsimd tensor ops (Pool only does residual-add STTs)
- additive causal mask applied via a PE matmul (lhsT=I, rhs=mask)
- rank-1 bias adds (bv/bo/b2) via K=1 matmuls into PSUM
- LN rstd = exp(-0.5*ln(var+eps)) so the scalar engine stays on ONE
  activation table (ln/exp/relu/copy/identity) -> no table swaps
- one DMA in + one DMA out per 512-token tile
"""

import sys

for p in ("/opt/trn_rl_repo",):
    if p not in sys.path:
        sys.path.insert(0, p)

import os as _os
from contextlib import ExitStack

import numpy as np

import concourse.bass as bass
import concourse.tile as tile
from concourse import bacc
from concourse import mybir
from concourse.bass_utils import run_bass_kernel_spmd

# Model dims (hardcoded per spec)
B, T, C, H = 8192, 16, 256, 4
HS = C // H          # 64
FF = 4 * C           # 1024
EPS = 1e-5
NCORES = 8
NTOK = (B // NCORES) * T   # 16384 tokens per core
TT = 512                   # tokens per big tile
NST = TT // 128            # 4 subtiles of 128 tokens
NTILES = NTOK // TT        # 32
CINV = float(C) ** -0.5
NEG = -60000.0             # fp16-representable -inf for the causal mask

F32 = mybir.dt.float32
F16 = mybir.dt.float16
AF = mybir.ActivationFunctionType
ALU = mybir.AluOpType
AX = mybir.AxisListType


def flat(ap, n):
    """View the first n contiguous free elements of a tile as [128, n]."""
    return bass.AP(tensor=ap.tensor, offset=ap.offset,
                   ap=[list(ap.ap[0]), [1, n]])


def dram_view(d_full, it):
    """[128, NST, C] view of tile `it` of a [NTOK, C] dram tensor."""
    return bass.AP(tensor=d_full.tensor, offset=d_full.offset + it * TT * C,
                   ap=[[C, 128], [128 * C, NST], [1, C]])


class TileCtx:
    """TileContext wrapper carrying an ExitStack for pools."""

    def __init__(self, nc):
        self._tc = tile.TileContext(nc)
        self._ctx = ExitStack()

    def __enter__(self):
        tc = self._tc.__enter__()
        tc._ctx = self._ctx
        return tc

    def __exit__(self, *a):
        self._ctx.close()
        return self._tc.__exit__(*a)


def build_kernel():
    nc = bacc.Bacc(None)

    x_d = nc.declare_dram_parameter("x", [NTOK, C], F32, isOutput=False)
    wqkv_d = nc.declare_dram_parameter("wqkv", [C, 3 * C], F16, isOutput=False)
    bqk_d = nc.declare_dram_parameter("bqk", [128, 4], F32, isOutput=False)
    wo_d = nc.declare_dram_parameter("wo", [C, C], F16, isOutput=False)
    w1_d = nc.declare_dram_parameter("w1", [C, FF], F16, isOutput=False)
    b1m_d = nc.declare_dram_parameter("b1m", [128, FF // 128], F32, isOutput=False)
    w2_d = nc.declare_dram_parameter("w2", [FF, C], F16, isOutput=False)
    brows_d = nc.declare_dram_parameter("brows", [128, C], F16, isOutput=False)
    mask4_d = nc.declare_dram_parameter("mask4", [128, H * 128], F16, isOutput=False)
    id_d = nc.declare_dram_parameter("ident", [128, 128], F16, isOutput=False)
    out_d = nc.declare_dram_parameter("out", [NTOK, C], F32, isOutput=True)

    x_full = x_d[:, :]
    out_full = out_d[:, :]

    with TileCtx(nc) as tc:
        ctx = tc._ctx
        singles = ctx.enter_context(tc.tile_pool(name="singles", bufs=1))
        xio = ctx.enter_context(tc.tile_pool(name="xio", bufs=2))
        hp = ctx.enter_context(tc.tile_pool(name="hp", bufs=2))
        qkp = ctx.enter_context(tc.tile_pool(name="qkp", bufs=2))
        ap_ = ctx.enter_context(tc.tile_pool(name="attn", bufs=2))
        mp = ctx.enter_context(tc.tile_pool(name="mp", bufs=2))
        sp = ctx.enter_context(tc.tile_pool(name="sp", bufs=6))
        psB = ctx.enter_context(tc.tile_pool(name="psB", bufs=3, space="PSUM"))
        psS = ctx.enter_context(tc.tile_pool(name="psS", bufs=3, space="PSUM"))
        psA = ctx.enter_context(tc.tile_pool(name="psA", bufs=1, space="PSUM"))

        # ---- persistent weights/constants ----
        wqkv_sb = []
        for k in range(2):
            t_ = singles.tile([128, 3 * C], F16, tag=f"wqkv{k}", name=f"wqkv{k}")
            nc.sync.dma_start(out=t_, in_=wqkv_d[k * 128:(k + 1) * 128, :])
            wqkv_sb.append(t_)
        wo_sb = []
        for k in range(2):
            t_ = singles.tile([128, C], F16, tag=f"wo{k}", name=f"wo{k}")
            nc.sync.dma_start(out=t_, in_=wo_d[k * 128:(k + 1) * 128, :])
            wo_sb.append(t_)
        w1_sb = []
        for k in range(2):
            t_ = singles.tile([128, FF], F16, tag=f"w1{k}", name=f"w1{k}")
            nc.sync.dma_start(out=t_, in_=w1_d[k * 128:(k + 1) * 128, :])
            w1_sb.append(t_)
        w2_sb = []
        for k in range(8):
            t_ = singles.tile([128, C], F16, tag=f"w2{k}", name=f"w2{k}")
            nc.sync.dma_start(out=t_, in_=w2_d[k * 128:(k + 1) * 128, :])
            w2_sb.append(t_)
        bqk_sb = singles.tile([128, 4], F32, tag="bqk", name="bqk")
        nc.sync.dma_start(out=bqk_sb, in_=bqk_d[:, :])
        b1m_sb = singles.tile([128, FF // 128], F32, tag="b1m", name="b1m")
        nc.sync.dma_start(out=b1m_sb, in_=b1m_d[:, :])
        brows_sb = singles.tile([128, C], F16, tag="brows", name="brows")
        nc.sync.dma_start(out=brows_sb, in_=brows_d[:, :])
        mask4_sb = singles.tile([128, H * 128], F16, tag="mask4", name="mask4")
        nc.sync.dma_start(out=mask4_sb, in_=mask4_d[:, :])
        id_sb = singles.tile([128, 128], F16, tag="ident", name="ident")
        nc.sync.dma_start(out=id_sb, in_=id_d[:, :])
        ones_sb = singles.tile([128, 128], F16, tag="ones", name="ones")
        nc.vector.memset(ones_sb, 1.0)
        # bias rows live at partitions 0 (bv), 32 (bo), 64 (b2) so the K=1
        # bias matmuls satisfy the base-partition-in-{0,32,64} constraint
        BV, BO, B2 = 0, 32, 64
        eps_sb = singles.tile([128, 1], F32, tag="eps", name="eps")
        nc.vector.memset(eps_sb, EPS)

        def layernorm_T(x_ap, hT_dst, st):
            """LN(x_ap) -> fp16, transposed into hT_dst[:, :, st*128:...]."""
            stats = sp.tile([128, 6], F32, tag="stats", name="stats")
            mv = sp.tile([128, 2], F32, tag="mv", name="mv")
            lnv = sp.tile([128, 1], F32, tag="lnv", name="lnv")
            rstd = sp.tile([128, 1], F32, tag="rstd", name="rstd")
            h_st = sp.tile([128, C], F16, tag="h", name="h")
            nc.vector.bn_stats(out=stats, in_=x_ap)
            nc.vector.bn_aggr(out=mv, in_=stats)
            nc.scalar.activation(out=lnv, in_=mv[:, 1:2], func=AF.Ln,
                                 bias=eps_sb, scale=1.0)
            nc.scalar.activation(out=rstd, in_=lnv, func=AF.Exp,
                                 bias=0.0, scale=-0.5)
            nc.vector.tensor_scalar(out=h_st, in0=x_ap,
                                    scalar1=mv[:, 0:1], scalar2=rstd,
                                    op0=ALU.subtract, op1=ALU.mult)
            tp = psS.tile([128, 4, 128], F32, tag="sm", name="htr")
            for cc in range(2):
                nc.tensor.matmul(out=tp[:, cc, :],
                                 lhsT=h_st[:, cc * 128:(cc + 1) * 128],
                                 rhs=id_sb, start=True, stop=True)
            nc.scalar.activation(out=hT_dst[:, :, st * 128:(st + 1) * 128],
                                 in_=tp[:, 0:2, :], func=AF.Copy,
                                 bias=0.0, scale=1.0)

        for it in range(NTILES):
            # ---- load x, LN1 -> hT (c-major fp16) ----
            x_sb = xio.tile([128, NST, C], F32, tag="x", name="x")
            nc.sync.dma_start(out=x_sb, in_=dram_view(x_full, it))
            hT = hp.tile([128, 2, TT], F16, tag="hT", name="hT")
            for st in range(NST):
                layernorm_T(x_sb[:, st, :], hT, st)

            # ---- QKV (q/k d-major with CINV+gamma folded; v token-major) ----
            qk_sb = [qkp.tile([128, TT], F16, tag=f"qk{m}", name=f"qk{m}")
                     for m in range(4)]
            for m in range(4):  # 0,1 q chunks; 2,3 k chunks
                ps = psB.tile([128, TT], F32, tag="big", name="qkps")
                for k in range(2):
                    nc.tensor.matmul(out=ps,
                                     lhsT=wqkv_sb[k][:, m * 128:(m + 1) * 128],
                                     rhs=flat(hT[:, k, :], TT),
                                     start=(k == 0), stop=(k == 1))
                nc.scalar.activation(out=qk_sb[m], in_=ps, func=AF.Identity,
                                     bias=bqk_sb[:, m:m + 1], scale=1.0)
            v_sb = ap_.tile([128, NST, C], F16, tag="v", name="v")
            for st in range(NST):
                psv = psS.tile([128, 4, 128], F32, tag="sm", name="vps")
                fv = flat(psv, C)
                nc.tensor.matmul(out=fv, lhsT=ones_sb[BV:BV + 1, :],
                                 rhs=brows_sb[BV:BV + 1, :],
                                 start=True, stop=False)
                for k in range(2):
                    nc.tensor.matmul(out=fv,
                                     lhsT=hT[:, k, st * 128:(st + 1) * 128],
                                     rhs=wqkv_sb[k][:, 2 * C:3 * C],
                                     start=False, stop=(k == 1))
                nc.vector.tensor_copy(out=v_sb[:, st, :], in_=fv)

            # ---- attention ----
            attnT_ps = psA.tile([128, 2, TT], F32, tag="attnT", name="attnT_ps")
            for st in range(NST):
                sl = slice(st * 128, (st + 1) * 128)
                ps_sc = psB.tile([128, H, 128], F32, tag="big", name="scps")
                for h in range(H):
                    hc, off = h // 2, (h % 2) * 64
                    nc.tensor.matmul(out=ps_sc[:, h, :], lhsT=id_sb,
                                     rhs=mask4_sb[:, h * 128:(h + 1) * 128],
                                     start=True, stop=False)
                    nc.tensor.matmul(out=ps_sc[:, h, :],
                                     lhsT=qk_sb[hc][off:off + 64, sl],
                                     rhs=qk_sb[2 + hc][off:off + 64, sl],
                                     start=False, stop=True)
                wn = ap_.tile([128, H, 128], F16, tag="wn", name="wn")
                nc.scalar.activation(out=wn, in_=ps_sc, func=AF.Exp,
                                     bias=0.0, scale=1.0)
                rsum = sp.tile([128, H], F32, tag="rsum", name="rsum")
                nc.vector.tensor_reduce(out=rsum, in_=wn, axis=AX.X, op=ALU.add)
                rcp = sp.tile([128, H], F32, tag="rcp", name="rcp")
                nc.vector.reciprocal(out=rcp, in_=rsum)
                wnn = ap_.tile([128, H, 128], F16, tag="wnn", name="wnn")
                for h in range(H):
                    nc.vector.tensor_scalar_mul(out=wnn[:, h, :],
                                                in0=wn[:, h, :],
                                                scalar1=rcp[:, h:h + 1])
                ps_wt = psB.tile([128, H, 128], F32, tag="big", name="wtps")
                for h in range(H):
                    nc.tensor.matmul(out=ps_wt[:, h, :], lhsT=wnn[:, h, :],
                                     rhs=id_sb, start=True, stop=True)
                wT = ap_.tile([128, H, 128], F16, tag="wT", name="wT")
                nc.scalar.activation(out=wT, in_=ps_wt, func=AF.Copy,
                                     bias=0.0, scale=1.0)
                for h in range(H):
                    hc, off = h // 2, (h % 2) * 64
                    nc.tensor.matmul(out=attnT_ps[off:off + 64, hc, sl],
                                     lhsT=v_sb[:, st, h * 64:(h + 1) * 64],
                                     rhs=wT[:, h, :], start=True, stop=True)
            attnT_sb = ap_.tile([128, 2, TT], F16, tag="attnT", name="attnT")
            nc.scalar.activation(out=attnT_sb[:, 0, :], in_=attnT_ps[:, 0, :],
                                 func=AF.Copy, bias=0.0, scale=1.0)
            nc.vector.tensor_copy(out=attnT_sb[:, 1, :], in_=attnT_ps[:, 1, :])

            # ---- Wo + residual -> x2; LN2 -> h2T ----
            x2_sb = xio.tile([128, NST, C], F32, tag="x2", name="x2")
            h2T = hp.tile([128, 2, TT], F16, tag="h2T", name="h2T")
            for st in range(NST):
                sl = slice(st * 128, (st + 1) * 128)
                pso = psS.tile([128, 4, 128], F32, tag="sm", name="wops")
                fo = flat(pso, C)
                nc.tensor.matmul(out=fo, lhsT=ones_sb[BO:BO + 1, :],
                                 rhs=brows_sb[BO:BO + 1, :],
                                 start=True, stop=False)
                for hc in range(2):
                    nc.tensor.matmul(out=fo, lhsT=attnT_sb[:, hc, sl],
                                     rhs=wo_sb[hc], start=False, stop=(hc == 1))
                nc.vector.scalar_tensor_tensor(out=x2_sb[:, st, :], in0=fo,
                                               scalar=1.0, in1=x_sb[:, st, :],
                                               op0=ALU.mult, op1=ALU.add)
                layernorm_T(x2_sb[:, st, :], h2T, st)

            # ---- MLP ----
            m1_sb = mp.tile([128, FF // 128, TT], F16, tag="m1", name="m1")
            for mf in range(8):
                ps = psB.tile([128, TT], F32, tag="big", name="m1ps")
                for k in range(2):
                    nc.tensor.matmul(out=ps,
                                     lhsT=w1_sb[k][:, mf * 128:(mf + 1) * 128],
                                     rhs=flat(h2T[:, k, :], TT),
                                     start=(k == 0), stop=(k == 1))
                if mf % 2 == 0:
                    nc.scalar.activation(out=m1_sb[:, mf, :], in_=ps,
                                         func=AF.Relu,
                                         bias=b1m_sb[:, mf:mf + 1], scale=1.0)
                else:
                    nc.vector.tensor_scalar(out=m1_sb[:, mf, :], in0=ps,
                                            scalar1=b1m_sb[:, mf:mf + 1],
                                            scalar2=0.0,
                                            op0=ALU.add, op1=ALU.max)
            o_sb = xio.tile([128, NST, C], F32, tag="o", name="o")
            for st in range(NST):
                sl = slice(st * 128, (st + 1) * 128)
                ps2 = psS.tile([128, 4, 128], F32, tag="sm", name="m2ps")
                f2 = flat(ps2, C)
                nc.tensor.matmul(out=f2, lhsT=ones_sb[B2:B2 + 1, :],
                                 rhs=brows_sb[B2:B2 + 1, :],
                                 start=True, stop=False)
                for mf in range(8):
                    nc.tensor.matmul(out=f2, lhsT=m1_sb[:, mf, sl],
                                     rhs=w2_sb[mf], start=False, stop=(mf == 7))
                nc.vector.scalar_tensor_tensor(out=o_sb[:, st, :], in0=f2,
                                               scalar=1.0, in1=x2_sb[:, st, :],
                                               op0=ALU.mult, op1=ALU.add)
            nc.sync.dma_start(out=dram_view(out_full, it), in_=o_sb)
    nc.finalize()
    return nc


def _prep_weights(Wq, Wk, Wv, Wo, bo, W1, b1, W2, b2, g1, be1, g2, be2):
    f16 = np.float16
    f32 = np.float32
    Wqf = np.transpose(Wq, (1, 0, 2)).reshape(C, C)  # [c,(h,d)]
    Wkf = np.transpose(Wk, (1, 0, 2)).reshape(C, C)
    Wvf = np.transpose(Wv, (1, 0, 2)).reshape(C, C)
    g1c = g1[:, None]
    wqkv = np.concatenate([g1c * Wqf * CINV, g1c * Wkf, g1c * Wvf],
                          axis=1).astype(f16)
    bq = (be1 @ Wqf) * CINV
    bk = be1 @ Wkf
    bv = be1 @ Wvf
    bqk = np.stack([bq[:128], bq[128:], bk[:128], bk[128:]], axis=1).astype(f32)
    w1 = (g2[:, None] * W1).astype(f16)
    b1e = (b1 + be2 @ W1).astype(f32)
    b1m = np.ascontiguousarray(b1e.reshape(8, 128).T)  # [128, 8]
    brows = np.zeros((128, C), dtype=f16)  # rows 0/32/64 = bv/bo/b2
    brows[0] = bv.astype(f16)
    brows[32] = bo.astype(f16)
    brows[64] = b2.astype(f16)
    # additive causal mask [128, 4*128] fp16 (tiled over 4 heads)
    m = np.full((128, 128), NEG, dtype=f32)
    for j in range(128 // T):
        blk = np.triu(np.full((T, T), NEG, dtype=f32), 1)
        m[j * T:(j + 1) * T, j * T:(j + 1) * T] = blk
    mask4 = np.tile(m, (1, H)).astype(f16)
    return dict(wqkv=wqkv, bqk=bqk, wo=Wo.astype(f16), w1=w1, b1m=b1m,
                w2=W2.astype(f16), brows=brows, mask4=mask4,
                ident=np.eye(128, dtype=f16))


_CACHE = {}


def kernel(x, Wq, Wk, Wv, Wo, bo, W1, b1, W2, b2, g1, be1, g2, be2):
    x = np.asarray(x, dtype=np.float32)
    wts = _prep_weights(np.asarray(Wq), np.asarray(Wk), np.asarray(Wv),
                        np.asarray(Wo), np.asarray(bo), np.asarray(W1),
                        np.asarray(b1), np.asarray(W2), np.asarray(b2),
                        np.asarray(g1), np.asarray(be1), np.asarray(g2),
                        np.asarray(be2))
    if "nc" not in _CACHE:
        _CACHE["nc"] = build_kernel()
    nc = _CACHE["nc"]
    xs = x.reshape(NCORES, NTOK, C)
    in_maps = [dict(x=np.ascontiguousarray(xs[i]), **wts) for i in range(NCORES)]
    kres = run_bass_kernel_spmd(
        nc, in_maps, list(range(NCORES)),
        trace=_os.environ.get("KERNEL_TRACE", "0") == "1",
        tmpdir=_os.environ.get("KERNEL_TRACE_DIR") or None)
    _CACHE["last"] = kres
    res = kres.results
    out = np.stack([res[i]["out"] for i in range(NCORES)], axis=0)
    return out.reshape(B, T, C).astype(np.float32)


if __name__ == "__main__":
    nc = build_kernel()
    print("kernel traced OK")


# revision 26
# speedup vs baseline: 1.4413x; 1.2631x over previous
"""Trainium2 Bass kernel for nn_Block_47261820125190 (dense transformer block).

Contract: kernel(**inputs) takes FULL inputs (x [8192,16,256] + weights),
shards batch across 8 NeuronCores (data parallel), runs a fused Bass/Tile
kernel per core, returns FULL output [8192,16,256] fp32.

v3 design notes:
- fp16 operands for all matmuls; fp32 residual stream
- LN rstd = exp(-0.5*ln(var+eps)); the activation-table registry is
  narrowed during build so every scalar-engine function resolves to the
  single table containing {ln,exp,relu,copy,identity} -> no table swaps
- rstd batched across the 4 subtiles (one ln + one exp per LN phase)
- causal mask applied multiplicatively on DVE after exp (no PE mask mms)
- softmax normalize via one broadcast tensor_tensor multiply per subtile
- rank-1 bias matmuls only emitted when biases are nonzero
"""

import sys

for p in ("/opt/trn_rl_repo",):
    if p not in sys.path:
        sys.path.insert(0, p)

import os as _os
from contextlib import ExitStack

import numpy as np

import concourse.bass as bass
import concourse.tile as tile
from concourse import bacc
from concourse import mybir
from concourse.bass_utils import run_bass_kernel_spmd

# Model dims (hardcoded per spec)
B, T, C, H = 8192, 16, 256, 4
HS = C // H          # 64
FF = 4 * C           # 1024
EPS = 1e-5
NCORES = 8
NTOK = (B // NCORES) * T   # 16384 tokens per core
TT = 512                   # tokens per big tile
NST = TT // 128            # 4 subtiles of 128 tokens
NTILES = NTOK // TT        # 32
CINV = float(C) ** -0.5

F32 = mybir.dt.float32
F16 = mybir.dt.float16
MASK_MM = _os.environ.get("MASK_MM", "1") == "1"
NEG = -60000.0
AF = mybir.ActivationFunctionType
ALU = mybir.AluOpType
AX = mybir.AxisListType


def flat(ap, n):
    """View the first n contiguous free elements of a tile as [128, n]."""
    return bass.AP(tensor=ap.tensor, offset=ap.offset,
                   ap=[list(ap.ap[0]), [1, n]])


def re_ap(ap, extra_off, dims):
    """Custom free-dim AP on a tile, keeping its partition dim."""
    return bass.AP(tensor=ap.tensor, offset=ap.offset + extra_off,
                   ap=[list(ap.ap[0])] + [list(d) for d in dims])


def dram_view(d_full, it):
    """[128, NST, C] view of tile `it` of a [NTOK, C] dram tensor."""
    return bass.AP(tensor=d_full.tensor, offset=d_full.offset + it * TT * C,
                   ap=[[C, 128], [128 * C, NST], [1, C]])


class TileCtx:
    """TileContext wrapper carrying an ExitStack for pools."""

    def __init__(self, nc):
        self._tc = tile.TileContext(nc)
        self._ctx = ExitStack()

    def __enter__(self):
        tc = self._tc.__enter__()
        tc._ctx = self._ctx
        return tc

    def __exit__(self, *a):
        self._ctx.close()
        return self._tc.__exit__(*a)


def _narrow_act_tables(arch):
    """Make every activation func we use resolve to the one table that
    contains them all, so no ACT_TABLE_LOAD swaps are emitted. Mutates the
    functools-cached dict; caller must cache_clear() afterwards."""
    from concourse.hw_specs import get_activation_tables
    tabs = get_activation_tables(arch)
    target = "natural_log_exp_and_others"
    need = {AF.Exp, AF.Ln, AF.Relu, AF.Copy, AF.Identity}
    if target in tabs and need <= tabs[target]:
        for name, funcs in tabs.items():
            if name != target:
                funcs -= need
    return get_activation_tables


def build_kernel(use_bias_mms=True):
    nc = bacc.Bacc(None)

    x_d = nc.declare_dram_parameter("x", [NTOK, C], F32, isOutput=False)
    wqkv_d = nc.declare_dram_parameter("wqkv", [C, 3 * C], F16, isOutput=False)
    bqk_d = nc.declare_dram_parameter("bqk", [128, 4], F32, isOutput=False)
    wo_d = nc.declare_dram_parameter("wo", [C, C], F16, isOutput=False)
    w1_d = nc.declare_dram_parameter("w1", [C, FF], F16, isOutput=False)
    b1m_d = nc.declare_dram_parameter("b1m", [128, FF // 128], F32, isOutput=False)
    w2_d = nc.declare_dram_parameter("w2", [FF, C], F16, isOutput=False)
    brows_d = nc.declare_dram_parameter("brows", [128, C], F16, isOutput=False)
    mask4_d = nc.declare_dram_parameter("mask4", [128, H * 128], F16, isOutput=False)
    id_d = nc.declare_dram_parameter("ident", [128, 128], F16, isOutput=False)
    out_d = nc.declare_dram_parameter("out", [NTOK, C], F32, isOutput=True)

    x_full = x_d[:, :]
    out_full = out_d[:, :]

    with TileCtx(nc) as tc:
        ctx = tc._ctx
        singles = ctx.enter_context(tc.tile_pool(name="singles", bufs=1))
        xio = ctx.enter_context(tc.tile_pool(name="xio", bufs=2))
        hp = ctx.enter_context(tc.tile_pool(name="hp", bufs=2))
        qkp = ctx.enter_context(tc.tile_pool(name="qkp", bufs=2))
        ap_ = ctx.enter_context(tc.tile_pool(name="attn", bufs=2))
        mp = ctx.enter_context(tc.tile_pool(name="mp", bufs=2))
        sp = ctx.enter_context(tc.tile_pool(name="sp", bufs=6))
        psB = ctx.enter_context(tc.tile_pool(name="psB", bufs=3, space="PSUM"))
        psS = ctx.enter_context(tc.tile_pool(name="psS", bufs=3, space="PSUM"))
        psA = ctx.enter_context(tc.tile_pool(name="psA", bufs=1, space="PSUM"))

        # ---- persistent weights/constants ----
        wqkv_sb = []
        for k in range(2):
            t_ = singles.tile([128, 3 * C], F16, tag=f"wqkv{k}", name=f"wqkv{k}")
            nc.sync.dma_start(out=t_, in_=wqkv_d[k * 128:(k + 1) * 128, :])
            wqkv_sb.append(t_)
        wo_sb = []
        for k in range(2):
            t_ = singles.tile([128, C], F16, tag=f"wo{k}", name=f"wo{k}")
            nc.sync.dma_start(out=t_, in_=wo_d[k * 128:(k + 1) * 128, :])
            wo_sb.append(t_)
        w1_sb = []
        for k in range(2):
            t_ = singles.tile([128, FF], F16, tag=f"w1{k}", name=f"w1{k}")
            nc.sync.dma_start(out=t_, in_=w1_d[k * 128:(k + 1) * 128, :])
            w1_sb.append(t_)
        w2_sb = []
        for k in range(8):
            t_ = singles.tile([128, C], F16, tag=f"w2{k}", name=f"w2{k}")
            nc.sync.dma_start(out=t_, in_=w2_d[k * 128:(k + 1) * 128, :])
            w2_sb.append(t_)
        bqk_sb = singles.tile([128, 4], F32, tag="bqk", name="bqk")
        nc.sync.dma_start(out=bqk_sb, in_=bqk_d[:, :])
        b1m_sb = singles.tile([128, FF // 128], F32, tag="b1m", name="b1m")
        nc.sync.dma_start(out=b1m_sb, in_=b1m_d[:, :])
        brows_sb = singles.tile([128, C], F16, tag="brows", name="brows")
        nc.sync.dma_start(out=brows_sb, in_=brows_d[:, :])
        mask4_sb = singles.tile([128, H * 128], F16, tag="mask4", name="mask4")
        nc.sync.dma_start(out=mask4_sb, in_=mask4_d[:, :])
        id_sb = singles.tile([128, 128], F16, tag="ident", name="ident")
        nc.sync.dma_start(out=id_sb, in_=id_d[:, :])
        ones_sb = singles.tile([128, 128], F16, tag="ones", name="ones")
        nc.vector.memset(ones_sb, 1.0)
        eps_sb = singles.tile([128, 1], F32, tag="eps", name="eps")
        nc.vector.memset(eps_sb, EPS)
        # bias rows live at partitions 0 (bv), 32 (bo), 64 (b2)
        BV, BO, B2 = 0, 32, 64

        def ln_phase(x_sb, hT):
            """LayerNorm all 4 subtiles of x_sb -> transposed fp16 hT."""
            mv4 = sp.tile([128, 2, NST], F32, tag="mv4", name="mv4")
            for st in range(NST):
                stats = sp.tile([128, 6], F32, tag="stats", name="stats")
                nc.vector.bn_stats(out=stats, in_=x_sb[:, st, :])
                nc.vector.bn_aggr(out=re_ap(mv4, st, [[NST, 2]]), in_=stats)
            lnv4 = sp.tile([128, NST], F32, tag="lnv4", name="lnv4")
            rstd4 = sp.tile([128, NST], F32, tag="rstd4", name="rstd4")
            nc.scalar.activation(out=lnv4, in_=mv4[:, 1, :], func=AF.Ln,
                                 bias=eps_sb, scale=1.0)
            nc.scalar.activation(out=rstd4, in_=lnv4, func=AF.Exp,
                                 bias=0.0, scale=-0.5)
            for st in range(NST):
                tp = psS.tile([128, 4, 128], F32, tag="sm", name="htr")
                h_st = sp.tile([128, C], F16, tag="h", name="h")
                nc.vector.tensor_scalar(out=h_st, in0=x_sb[:, st, :],
                                        scalar1=mv4[:, 0, st:st + 1],
                                        scalar2=rstd4[:, st:st + 1],
                                        op0=ALU.subtract, op1=ALU.mult)
                for cc in range(2):
                    nc.tensor.matmul(out=tp[:, cc, :],
                                     lhsT=h_st[:, cc * 128:(cc + 1) * 128],
                                     rhs=id_sb, start=True, stop=True)
                nc.scalar.activation(
                    out=hT[:, :, st * 128:(st + 1) * 128],
                    in_=tp[:, 0:2, :], func=AF.Copy, bias=0.0, scale=1.0)

        for it in range(NTILES):
            # ---- load x, LN1 -> hT (c-major fp16) ----
            x_sb = xio.tile([128, NST, C], F32, tag="x", name="x")
            nc.sync.dma_start(out=x_sb, in_=dram_view(x_full, it))
            hT = hp.tile([128, 2, TT], F16, tag="hT", name="hT")
            ln_phase(x_sb, hT)

            # ---- QKV (q/k d-major with CINV+gamma folded; v token-major) ----
            qk_sb = [qkp.tile([128, TT], F16, tag=f"qk{m}", name=f"qk{m}")
                     for m in range(4)]
            for m in range(4):  # 0,1 q chunks; 2,3 k chunks
                ps = psB.tile([128, TT], F32, tag="big", name="qkps")
                for k in range(2):
                    nc.tensor.matmul(out=ps,
                                     lhsT=wqkv_sb[k][:, m * 128:(m + 1) * 128],
                                     rhs=flat(hT[:, k, :], TT),
                                     start=(k == 0), stop=(k == 1))
                nc.scalar.activation(out=qk_sb[m], in_=ps, func=AF.Identity,
                                     bias=bqk_sb[:, m:m + 1], scale=1.0)
            v_sb = ap_.tile([128, NST, C], F16, tag="v", name="v")
            for st in range(NST):
                psv = psS.tile([128, 4, 128], F32, tag="sm", name="vps")
                fv = flat(psv, C)
                if use_bias_mms:
                    nc.tensor.matmul(out=fv, lhsT=ones_sb[BV:BV + 1, :],
                                     rhs=brows_sb[BV:BV + 1, :],
                                     start=True, stop=False)
                for k in range(2):
                    nc.tensor.matmul(out=fv,
                                     lhsT=hT[:, k, st * 128:(st + 1) * 128],
                                     rhs=wqkv_sb[k][:, 2 * C:3 * C],
                                     start=(not use_bias_mms and k == 0),
                                     stop=(k == 1))
                nc.vector.tensor_copy(out=v_sb[:, st, :], in_=fv)

            # ---- attention ----
            attnT_ps = psA.tile([128, 2, TT], F32, tag="attnT", name="attnT_ps")
            for st in range(NST):
                sl = slice(st * 128, (st + 1) * 128)
                ps_sc = psB.tile([128, H, 128], F32, tag="big", name="scps")
                if MASK_MM:
                    for h in range(H):
                        hc, off = h // 2, (h % 2) * 64
                        nc.tensor.matmul(out=ps_sc[:, h, :], lhsT=id_sb,
                                         rhs=mask4_sb[:, h * 128:(h + 1) * 128],
                                         start=True, stop=False)
                        nc.tensor.matmul(out=ps_sc[:, h, :],
                                         lhsT=qk_sb[hc][off:off + 64, sl],
                                         rhs=qk_sb[2 + hc][off:off + 64, sl],
                                         start=False, stop=True)
                else:
                    for h in range(H):
                        hc, off = h // 2, (h % 2) * 64
                        nc.tensor.matmul(out=ps_sc[:, h, :],
                                         lhsT=qk_sb[hc][off:off + 64, sl],
                                         rhs=qk_sb[2 + hc][off:off + 64, sl],
                                         start=True, stop=True)
                wn = ap_.tile([128, H, 128], F16, tag="wn", name="wn")
                nc.scalar.activation(out=wn, in_=ps_sc, func=AF.Exp,
                                     bias=0.0, scale=1.0)
                if MASK_MM:
                    wnm = wn
                else:
                    wnm = ap_.tile([128, H, 128], F16, tag="wnm", name="wnm")
                    nc.vector.tensor_tensor(
                        out=wnm, in0=wn,
                        in1=re_ap(mask4_sb, 0, [[128, H], [1, 128]]),
                        op=ALU.mult)
                rsum = sp.tile([128, H], F32, tag="rsum", name="rsum")
                nc.vector.tensor_reduce(out=rsum, in_=wnm, axis=AX.X, op=ALU.add)
                rcp = sp.tile([128, H], F32, tag="rcp", name="rcp")
                nc.vector.reciprocal(out=rcp, in_=rsum)
                wnn = ap_.tile([128, H, 128], F16, tag="wnn", name="wnn")
                for h in range(H):
                    nc.vector.tensor_scalar_mul(out=wnn[:, h, :],
                                                in0=wnm[:, h, :],
                                                scalar1=rcp[:, h:h + 1])
                ps_wt = psB.tile([128, H, 128], F32, tag="big", name="wtps")
                for h in range(H):
                    nc.tensor.matmul(out=ps_wt[:, h, :], lhsT=wnn[:, h, :],
                                     rhs=id_sb, start=True, stop=True)
                wT = ap_.tile([128, H, 128], F16, tag="wT", name="wT")
                nc.scalar.activation(out=wT, in_=ps_wt, func=AF.Copy,
                                     bias=0.0, scale=1.0)
                for h in range(H):
                    hc, off = h // 2, (h % 2) * 64
                    nc.tensor.matmul(out=attnT_ps[off:off + 64, hc, sl],
                                     lhsT=v_sb[:, st, h * 64:(h + 1) * 64],
                                     rhs=wT[:, h, :], start=True, stop=True)
            attnT_sb = ap_.tile([128, 2, TT], F16, tag="attnT", name="attnT")
            nc.scalar.activation(out=attnT_sb[:, 0, :], in_=attnT_ps[:, 0, :],
                                 func=AF.Copy, bias=0.0, scale=1.0)
            nc.vector.tensor_copy(out=attnT_sb[:, 1, :], in_=attnT_ps[:, 1, :])

            # ---- Wo + residual -> x2; LN2 -> h2T ----
            x2_sb = xio.tile([128, NST, C], F32, tag="x2", name="x2")
            for st in range(NST):
                sl = slice(st * 128, (st + 1) * 128)
                pso = psS.tile([128, 4, 128], F32, tag="sm", name="wops")
                fo = flat(pso, C)
                if use_bias_mms:
                    nc.tensor.matmul(out=fo, lhsT=ones_sb[BO:BO + 1, :],
                                     rhs=brows_sb[BO:BO + 1, :],
                                     start=True, stop=False)
                for hc in range(2):
                    nc.tensor.matmul(out=fo, lhsT=attnT_sb[:, hc, sl],
                                     rhs=wo_sb[hc],
                                     start=(not use_bias_mms and hc == 0),
                                     stop=(hc == 1))
                nc.vector.scalar_tensor_tensor(out=x2_sb[:, st, :], in0=fo,
                                               scalar=1.0, in1=x_sb[:, st, :],
                                               op0=ALU.mult, op1=ALU.add)
            h2T = hp.tile([128, 2, TT], F16, tag="h2T", name="h2T")
            ln_phase(x2_sb, h2T)

            # ---- MLP ----
            m1_sb = mp.tile([128, FF // 128, TT], F16, tag="m1", name="m1")
            for mf in range(8):
                ps = psB.tile([128, TT], F32, tag="big", name="m1ps")
                for k in range(2):
                    nc.tensor.matmul(out=ps,
                                     lhsT=w1_sb[k][:, mf * 128:(mf + 1) * 128],
                                     rhs=flat(h2T[:, k, :], TT),
                                     start=(k == 0), stop=(k == 1))
                nc.scalar.activation(out=m1_sb[:, mf, :], in_=ps,
                                     func=AF.Relu,
                                     bias=b1m_sb[:, mf:mf + 1], scale=1.0)
            o_sb = xio.tile([128, NST, C], F32, tag="o", name="o")
            for st in range(NST):
                sl = slice(st * 128, (st + 1) * 128)
                ps2 = psS.tile([128, 4, 128], F32, tag="sm", name="m2ps")
                f2 = flat(ps2, C)
                if use_bias_mms:
                    nc.tensor.matmul(out=f2, lhsT=ones_sb[B2:B2 + 1, :],
                                     rhs=brows_sb[B2:B2 + 1, :],
                                     start=True, stop=False)
                for mf in range(8):
                    nc.tensor.matmul(out=f2, lhsT=m1_sb[:, mf, sl],
                                     rhs=w2_sb[mf],
                                     start=(not use_bias_mms and mf == 0),
                                     stop=(mf == 7))
                nc.vector.scalar_tensor_tensor(out=o_sb[:, st, :], in0=f2,
                                               scalar=1.0, in1=x2_sb[:, st, :],
                                               op0=ALU.mult, op1=ALU.add)
            nc.sync.dma_start(out=dram_view(out_full, it), in_=o_sb)

    if _os.environ.get("NO_ACT_NARROW", "0") == "1":
        nc.finalize()
    else:
        gat = _narrow_act_tables(nc.m.arch)
        try:
            nc.finalize()
        finally:
            gat.cache_clear()
    return nc


def _prep_weights(Wq, Wk, Wv, Wo, bo, W1, b1, W2, b2, g1, be1, g2, be2):
    f16 = np.float16
    f32 = np.float32
    Wqf = np.transpose(Wq, (1, 0, 2)).reshape(C, C)  # [c,(h,d)]
    Wkf = np.transpose(Wk, (1, 0, 2)).reshape(C, C)
    Wvf = np.transpose(Wv, (1, 0, 2)).reshape(C, C)
    g1c = g1[:, None]
    wqkv = np.concatenate([g1c * Wqf * CINV, g1c * Wkf, g1c * Wvf],
                          axis=1).astype(f16)
    bq = (be1 @ Wqf) * CINV
    bk = be1 @ Wkf
    bv = be1 @ Wvf
    bqk = np.stack([bq[:128], bq[128:], bk[:128], bk[128:]], axis=1).astype(f32)
    w1 = (g2[:, None] * W1).astype(f16)
    b1e = (b1 + be2 @ W1).astype(f32)
    b1m = np.ascontiguousarray(b1e.reshape(8, 128).T)  # [128, 8]
    brows = np.zeros((128, C), dtype=f16)  # rows 0/32/64 = bv/bo/b2
    brows[0] = bv.astype(f16)
    brows[32] = bo.astype(f16)
    brows[64] = b2.astype(f16)
    # causal mask [128, 4*128] fp16 (tiled over 4 heads):
    # multiplicative 0/1 by default, additive 0/NEG when MASK_MM
    if MASK_MM:
        m = np.full((128, 128), NEG, dtype=f32)
        for j in range(128 // T):
            m[j * T:(j + 1) * T, j * T:(j + 1) * T] = np.triu(
                np.full((T, T), NEG, dtype=f32), 1)
    else:
        m = np.zeros((128, 128), dtype=f32)
        for j in range(128 // T):
            m[j * T:(j + 1) * T, j * T:(j + 1) * T] = np.tril(
                np.ones((T, T), dtype=f32))
    mask4 = np.tile(m, (1, H)).astype(f16)
    return dict(wqkv=wqkv, bqk=bqk, wo=Wo.astype(f16), w1=w1, b1m=b1m,
                w2=W2.astype(f16), brows=brows, mask4=mask4,
                ident=np.eye(128, dtype=f16))


_CACHE = {}


def kernel(x, Wq, Wk, Wv, Wo, bo, W1, b1, W2, b2, g1, be1, g2, be2):
    x = np.asarray(x, dtype=np.float32)
    wts = _prep_weights(np.asarray(Wq), np.asarray(Wk), np.asarray(Wv),
                        np.asarray(Wo), np.asarray(bo), np.asarray(W1),
                        np.asarray(b1), np.asarray(W2), np.asarray(b2),
                        np.asarray(g1), np.asarray(be1), np.asarray(g2),
                        np.asarray(be2))
    use_bias = bool(np.any(wts["brows"])) or \
        _os.environ.get("FORCE_BIAS", "0") == "1"
    key = ("nc", use_bias)
    if key not in _CACHE:
        _CACHE[key] = build_kernel(use_bias_mms=use_bias)
    nc = _CACHE[key]
    xs = x.reshape(NCORES, NTOK, C)
    in_maps = [dict(x=np.ascontiguousarray(xs[i]), **wts) for i in range(NCORES)]
    kres = run_bass_kernel_spmd(
        nc, in_maps, list(range(NCORES)),
        trace=_os.environ.get("KERNEL_TRACE", "0") == "1",
        tmpdir=_os.environ.get("KERNEL_TRACE_DIR") or None)
    _CACHE["last"] = kres
    res = kres.results
    out = np.stack([res[i]["out"] for i in range(NCORES)], axis=0)
    return out.reshape(B, T, C).astype(np.float32)


if __name__ == "__main__":
    nc = build_kernel()
    print("kernel traced OK")


# revision 32
# speedup vs baseline: 2.0565x; 1.4268x over previous
"""Trainium2 Bass kernel for nn_Block_47261820125190 (dense transformer block).

Contract: kernel(**inputs) takes FULL inputs (x [8192,16,256] + weights),
shards batch across 8 NeuronCores (data parallel), runs a fused Bass/Tile
kernel per core, returns FULL output [8192,16,256] fp32.

v3 design notes:
- fp16 operands for all matmuls; fp32 residual stream
- LN rstd = exp(-0.5*ln(var+eps)); the activation-table registry is
  narrowed during build so every scalar-engine function resolves to the
  single table containing {ln,exp,relu,copy,identity} -> no table swaps
- rstd batched across the 4 subtiles (one ln + one exp per LN phase)
- causal mask applied multiplicatively on DVE after exp (no PE mask mms)
- softmax normalize via one broadcast tensor_tensor multiply per subtile
- rank-1 bias matmuls only emitted when biases are nonzero
"""

import sys

for p in ("/opt/trn_rl_repo",):
    if p not in sys.path:
        sys.path.insert(0, p)

import os as _os
from contextlib import ExitStack

import numpy as np

import concourse.bass as bass
import concourse.tile as tile
from concourse import bacc
from concourse import mybir
from concourse.bass_utils import run_bass_kernel_spmd

# Model dims (hardcoded per spec)
B, T, C, H = 8192, 16, 256, 4
HS = C // H          # 64
FF = 4 * C           # 1024
EPS = 1e-5
NCORES = 8
NTOK = (B // NCORES) * T   # 16384 tokens per core
TT = 512                   # tokens per big tile
NST = TT // 128            # 4 subtiles of 128 tokens
NTILES = NTOK // TT        # 32
CINV = float(C) ** -0.5

F32 = mybir.dt.float32
F16 = mybir.dt.float16
MASK_MM = _os.environ.get("MASK_MM", "1") == "1"
NEG = -60000.0
AF = mybir.ActivationFunctionType
ALU = mybir.AluOpType
AX = mybir.AxisListType


def flat(ap, n):
    """View the first n contiguous free elements of a tile as [128, n]."""
    return bass.AP(tensor=ap.tensor, offset=ap.offset,
                   ap=[list(ap.ap[0]), [1, n]])


def re_ap(ap, extra_off, dims):
    """Custom free-dim AP on a tile, keeping its partition dim."""
    return bass.AP(tensor=ap.tensor, offset=ap.offset + extra_off,
                   ap=[list(ap.ap[0])] + [list(d) for d in dims])


def dram_view(d_full, it):
    """[128, NST, C] view of tile `it` of a [NTOK, C] dram tensor."""
    return bass.AP(tensor=d_full.tensor, offset=d_full.offset + it * TT * C,
                   ap=[[C, 128], [128 * C, NST], [1, C]])


class TileCtx:
    """TileContext wrapper carrying an ExitStack for pools."""

    def __init__(self, nc):
        self._tc = tile.TileContext(nc)
        self._ctx = ExitStack()

    def __enter__(self):
        tc = self._tc.__enter__()
        tc._ctx = self._ctx
        return tc

    def __exit__(self, *a):
        self._ctx.close()
        return self._tc.__exit__(*a)


def _narrow_act_tables(arch):
    """Make every activation func we use resolve to the one table that
    contains them all, so no ACT_TABLE_LOAD swaps are emitted. Mutates the
    functools-cached dict; caller must cache_clear() afterwards."""
    from concourse.hw_specs import get_activation_tables
    tabs = get_activation_tables(arch)
    target = "natural_log_exp_and_others"
    need = {AF.Exp, AF.Ln, AF.Relu, AF.Copy, AF.Identity}
    if target in tabs and need <= tabs[target]:
        for name, funcs in tabs.items():
            if name != target:
                funcs -= need
    return get_activation_tables


def build_kernel(use_bias_mms=True):
    nc = bacc.Bacc(None)

    x_d = nc.declare_dram_parameter("x", [NTOK, C], F32, isOutput=False)
    wqkv_d = nc.declare_dram_parameter("wqkv", [C, 3 * C], F16, isOutput=False)
    bqk_d = nc.declare_dram_parameter("bqk", [128, 4], F32, isOutput=False)
    wo_d = nc.declare_dram_parameter("wo", [C, C], F16, isOutput=False)
    w1_d = nc.declare_dram_parameter("w1", [C, FF], F16, isOutput=False)
    b1m_d = nc.declare_dram_parameter("b1m", [128, FF // 128], F32, isOutput=False)
    w2_d = nc.declare_dram_parameter("w2", [FF, C], F16, isOutput=False)
    brows_d = nc.declare_dram_parameter("brows", [128, C], F16, isOutput=False)
    mask4_d = nc.declare_dram_parameter("mask4", [128, H * 128], F16, isOutput=False)
    id_d = nc.declare_dram_parameter("ident", [128, 128], F16, isOutput=False)
    out_d = nc.declare_dram_parameter("out", [NTOK, C], F32, isOutput=True)

    x_full = x_d[:, :]
    out_full = out_d[:, :]

    with TileCtx(nc) as tc:
        ctx = tc._ctx
        singles = ctx.enter_context(tc.tile_pool(name="singles", bufs=1))
        xio = ctx.enter_context(tc.tile_pool(name="xio", bufs=2))
        hp = ctx.enter_context(tc.tile_pool(name="hp", bufs=2))
        qkp = ctx.enter_context(tc.tile_pool(name="qkp", bufs=2))
        ap_ = ctx.enter_context(tc.tile_pool(name="attn", bufs=2))
        mp = ctx.enter_context(tc.tile_pool(name="mp", bufs=2))
        sp = ctx.enter_context(tc.tile_pool(name="sp", bufs=6))
        psB = ctx.enter_context(tc.tile_pool(name="psB", bufs=3, space="PSUM"))
        psS = ctx.enter_context(tc.tile_pool(name="psS", bufs=3, space="PSUM"))
        psA = ctx.enter_context(tc.tile_pool(name="psA", bufs=1, space="PSUM"))

        # ---- persistent weights/constants ----
        wqkv_sb = []
        for k in range(2):
            t_ = singles.tile([128, 3 * C], F16, tag=f"wqkv{k}", name=f"wqkv{k}")
            nc.sync.dma_start(out=t_, in_=wqkv_d[k * 128:(k + 1) * 128, :])
            wqkv_sb.append(t_)
        wo_sb = []
        for k in range(2):
            t_ = singles.tile([128, C], F16, tag=f"wo{k}", name=f"wo{k}")
            nc.sync.dma_start(out=t_, in_=wo_d[k * 128:(k + 1) * 128, :])
            wo_sb.append(t_)
        w1_sb = []
        for k in range(2):
            t_ = singles.tile([128, FF], F16, tag=f"w1{k}", name=f"w1{k}")
            nc.sync.dma_start(out=t_, in_=w1_d[k * 128:(k + 1) * 128, :])
            w1_sb.append(t_)
        w2_sb = []
        for k in range(8):
            t_ = singles.tile([128, C], F16, tag=f"w2{k}", name=f"w2{k}")
            nc.sync.dma_start(out=t_, in_=w2_d[k * 128:(k + 1) * 128, :])
            w2_sb.append(t_)
        bqk_sb = singles.tile([128, 4], F32, tag="bqk", name="bqk")
        nc.sync.dma_start(out=bqk_sb, in_=bqk_d[:, :])
        b1m_sb = singles.tile([128, FF // 128], F32, tag="b1m", name="b1m")
        nc.sync.dma_start(out=b1m_sb, in_=b1m_d[:, :])
        brows_sb = singles.tile([128, C], F16, tag="brows", name="brows")
        nc.sync.dma_start(out=brows_sb, in_=brows_d[:, :])
        mask4_sb = singles.tile([128, H * 128], F16, tag="mask4", name="mask4")
        nc.sync.dma_start(out=mask4_sb, in_=mask4_d[:, :])
        id_sb = singles.tile([128, 128], F16, tag="ident", name="ident")
        nc.sync.dma_start(out=id_sb, in_=id_d[:, :])
        ones_sb = singles.tile([128, 128], F16, tag="ones", name="ones")
        nc.vector.memset(ones_sb, 1.0)
        eps_sb = singles.tile([128, 1], F32, tag="eps", name="eps")
        nc.vector.memset(eps_sb, EPS)
        # bias rows live at partitions 0 (bv), 32 (bo), 64 (b2)
        BV, BO, B2 = 0, 32, 64

        def ln_phase(x_sb, hT):
            """LayerNorm all 4 subtiles of x_sb -> transposed fp16 hT."""
            mv4 = sp.tile([128, 2, NST], F32, tag="mv4", name="mv4")
            for st in range(NST):
                stats = sp.tile([128, 6], F32, tag="stats", name="stats")
                nc.vector.bn_stats(out=stats, in_=x_sb[:, st, :])
                nc.vector.bn_aggr(out=re_ap(mv4, st, [[NST, 2]]), in_=stats)
            lnv4 = sp.tile([128, NST], F32, tag="lnv4", name="lnv4")
            rstd4 = sp.tile([128, NST], F32, tag="rstd4", name="rstd4")
            nc.scalar.activation(out=lnv4, in_=mv4[:, 1, :], func=AF.Ln,
                                 bias=eps_sb, scale=1.0)
            nc.scalar.activation(out=rstd4, in_=lnv4, func=AF.Exp,
                                 bias=0.0, scale=-0.5)
            hs = []
            for st in range(NST):
                h_st = sp.tile([128, C], F16, tag=f"h{st}", name=f"h{st}")
                nc.vector.tensor_scalar(out=h_st, in0=x_sb[:, st, :],
                                        scalar1=mv4[:, 0, st:st + 1],
                                        scalar2=rstd4[:, st:st + 1],
                                        op0=ALU.subtract, op1=ALU.mult)
                hs.append(h_st)
            tps = []
            for st in range(NST):
                tp = psS.tile([128, 4, 128], F32, tag="sm", name="htr")
                for cc in range(2):
                    nc.tensor.matmul(out=tp[:, cc, :],
                                     lhsT=hs[st][:, cc * 128:(cc + 1) * 128],
                                     rhs=id_sb, start=True, stop=True)
                tps.append(tp)
            for st in range(NST):
                nc.scalar.activation(
                    out=hT[:, :, st * 128:(st + 1) * 128],
                    in_=tps[st][:, 0:2, :], func=AF.Copy, bias=0.0, scale=1.0)

        for it in range(NTILES):
            # ---- load x, LN1 -> hT (c-major fp16) ----
            x_sb = xio.tile([128, NST, C], F32, tag="x", name="x")
            nc.sync.dma_start(out=x_sb, in_=dram_view(x_full, it))
            hT = hp.tile([128, 2, TT], F16, tag="hT", name="hT")
            ln_phase(x_sb, hT)

            # ---- QKV (q/k d-major with CINV+gamma folded; v token-major) ----
            qk_sb = [qkp.tile([128, TT], F16, tag=f"qk{m}", name=f"qk{m}")
                     for m in range(4)]
            for m in range(4):  # 0,1 q chunks; 2,3 k chunks
                ps = psB.tile([128, TT], F32, tag="big", name="qkps")
                for k in range(2):
                    nc.tensor.matmul(out=ps,
                                     lhsT=wqkv_sb[k][:, m * 128:(m + 1) * 128],
                                     rhs=flat(hT[:, k, :], TT),
                                     start=(k == 0), stop=(k == 1))
                nc.scalar.activation(out=qk_sb[m], in_=ps, func=AF.Identity,
                                     bias=bqk_sb[:, m:m + 1], scale=1.0)
            v_sb = ap_.tile([128, NST, C], F16, tag="v", name="v")
            vps = []
            for st in range(NST):
                psv = psS.tile([128, 4, 128], F32, tag="sm", name="vps")
                fv = flat(psv, C)
                if use_bias_mms:
                    nc.tensor.matmul(out=fv, lhsT=ones_sb[BV:BV + 1, :],
                                     rhs=brows_sb[BV:BV + 1, :],
                                     start=True, stop=False)
                for k in range(2):
                    nc.tensor.matmul(out=fv,
                                     lhsT=hT[:, k, st * 128:(st + 1) * 128],
                                     rhs=wqkv_sb[k][:, 2 * C:3 * C],
                                     start=(not use_bias_mms and k == 0),
                                     stop=(k == 1))
                vps.append(fv)
            for st in range(NST):
                nc.vector.tensor_copy(out=v_sb[:, st, :], in_=vps[st])

            # ---- attention (phase-batched so engines pipeline across sts) ----
            attnT_ps = psA.tile([128, 2, TT], F32, tag="attnT", name="attnT_ps")
            ps_scs = []
            for st in range(NST):
                sl = slice(st * 128, (st + 1) * 128)
                ps_sc = psB.tile([128, H, 128], F32, tag="big", name="scps")
                for h in range(H):
                    hc, off = h // 2, (h % 2) * 64
                    nc.tensor.matmul(out=ps_sc[:, h, :], lhsT=id_sb,
                                     rhs=mask4_sb[:, h * 128:(h + 1) * 128],
                                     start=True, stop=False)
                    nc.tensor.matmul(out=ps_sc[:, h, :],
                                     lhsT=qk_sb[hc][off:off + 64, sl],
                                     rhs=qk_sb[2 + hc][off:off + 64, sl],
                                     start=False, stop=True)
                ps_scs.append(ps_sc)
            wns = []
            for st in range(NST):
                wn = ap_.tile([128, H, 128], F16, tag=f"wn{st}", name=f"wn{st}")
                nc.scalar.activation(out=wn, in_=ps_scs[st], func=AF.Exp,
                                     bias=0.0, scale=1.0)
                wns.append(wn)
            wnns = []
            for st in range(NST):
                rsum = sp.tile([128, H], F32, tag="rsum", name="rsum")
                nc.vector.tensor_reduce(out=rsum, in_=wns[st], axis=AX.X,
                                        op=ALU.add)
                rcp = sp.tile([128, H], F32, tag="rcp", name="rcp")
                nc.vector.reciprocal(out=rcp, in_=rsum)
                wnn = ap_.tile([128, H, 128], F16, tag=f"wnn{st}",
                               name=f"wnn{st}")
                for h in range(H):
                    nc.vector.tensor_scalar_mul(out=wnn[:, h, :],
                                                in0=wns[st][:, h, :],
                                                scalar1=rcp[:, h:h + 1])
                wnns.append(wnn)
            ps_wts = []
            for st in range(NST):
                ps_wt = psS.tile([128, 4, 128], F32, tag="sm", name="wtps")
                for h in range(H):
                    nc.tensor.matmul(out=ps_wt[:, h, :], lhsT=wnns[st][:, h, :],
                                     rhs=id_sb, start=True, stop=True)
                ps_wts.append(ps_wt)
            wTs = []
            for st in range(NST):
                wT = ap_.tile([128, H, 128], F16, tag=f"wT{st}", name=f"wT{st}")
                nc.scalar.activation(out=wT, in_=ps_wts[st], func=AF.Copy,
                                     bias=0.0, scale=1.0)
                wTs.append(wT)
            for st in range(NST):
                sl = slice(st * 128, (st + 1) * 128)
                for h in range(H):
                    hc, off = h // 2, (h % 2) * 64
                    nc.tensor.matmul(out=attnT_ps[off:off + 64, hc, sl],
                                     lhsT=v_sb[:, st, h * 64:(h + 1) * 64],
                                     rhs=wTs[st][:, h, :], start=True, stop=True)
            attnT_sb = ap_.tile([128, 2, TT], F16, tag="attnT", name="attnT")
            nc.scalar.activation(out=attnT_sb[:, 0, :], in_=attnT_ps[:, 0, :],
                                 func=AF.Copy, bias=0.0, scale=1.0)
            nc.vector.tensor_copy(out=attnT_sb[:, 1, :], in_=attnT_ps[:, 1, :])

            # ---- Wo + residual -> x2; LN2 -> h2T ----
            x2_sb = xio.tile([128, NST, C], F32, tag="x2", name="x2")
            wops = []
            for st in range(NST):
                sl = slice(st * 128, (st + 1) * 128)
                pso = psS.tile([128, 4, 128], F32, tag="sm", name="wops")
                fo = flat(pso, C)
                if use_bias_mms:
                    nc.tensor.matmul(out=fo, lhsT=ones_sb[BO:BO + 1, :],
                                     rhs=brows_sb[BO:BO + 1, :],
                                     start=True, stop=False)
                for hc in range(2):
                    nc.tensor.matmul(out=fo, lhsT=attnT_sb[:, hc, sl],
                                     rhs=wo_sb[hc],
                                     start=(not use_bias_mms and hc == 0),
                                     stop=(hc == 1))
                wops.append(fo)
            for st in range(NST):
                nc.vector.scalar_tensor_tensor(out=x2_sb[:, st, :],
                                               in0=wops[st],
                                               scalar=1.0, in1=x_sb[:, st, :],
                                               op0=ALU.mult, op1=ALU.add)
            h2T = hp.tile([128, 2, TT], F16, tag="h2T", name="h2T")
            ln_phase(x2_sb, h2T)

            # ---- MLP ----
            m1_sb = mp.tile([128, FF // 128, TT], F16, tag="m1", name="m1")
            for mf in range(8):
                ps = psB.tile([128, TT], F32, tag="big", name="m1ps")
                for k in range(2):
                    nc.tensor.matmul(out=ps,
                                     lhsT=w1_sb[k][:, mf * 128:(mf + 1) * 128],
                                     rhs=flat(h2T[:, k, :], TT),
                                     start=(k == 0), stop=(k == 1))
                nc.scalar.activation(out=m1_sb[:, mf, :], in_=ps,
                                     func=AF.Relu,
                                     bias=b1m_sb[:, mf:mf + 1], scale=1.0)
            o_sb = xio.tile([128, NST, C], F32, tag="o", name="o")
            m2ps = []
            for st in range(NST):
                sl = slice(st * 128, (st + 1) * 128)
                ps2 = psS.tile([128, 4, 128], F32, tag="sm", name="m2ps")
                f2 = flat(ps2, C)
                if use_bias_mms:
                    nc.tensor.matmul(out=f2, lhsT=ones_sb[B2:B2 + 1, :],
                                     rhs=brows_sb[B2:B2 + 1, :],
                                     start=True, stop=False)
                for mf in range(8):
                    nc.tensor.matmul(out=f2, lhsT=m1_sb[:, mf, sl],
                                     rhs=w2_sb[mf],
                                     start=(not use_bias_mms and mf == 0),
                                     stop=(mf == 7))
                m2ps.append(f2)
            for st in range(NST):
                nc.vector.scalar_tensor_tensor(out=o_sb[:, st, :],
                                               in0=m2ps[st],
                                               scalar=1.0, in1=x2_sb[:, st, :],
                                               op0=ALU.mult, op1=ALU.add)
            nc.sync.dma_start(out=dram_view(out_full, it), in_=o_sb)

    if _os.environ.get("NO_ACT_NARROW", "0") == "1":
        nc.finalize()
    else:
        gat = _narrow_act_tables(nc.m.arch)
        try:
            nc.finalize()
        finally:
            gat.cache_clear()
    return nc


def _prep_weights(Wq, Wk, Wv, Wo, bo, W1, b1, W2, b2, g1, be1, g2, be2):
    f16 = np.float16
    f32 = np.float32
    Wqf = np.transpose(Wq, (1, 0, 2)).reshape(C, C)  # [c,(h,d)]
    Wkf = np.transpose(Wk, (1, 0, 2)).reshape(C, C)
    Wvf = np.transpose(Wv, (1, 0, 2)).reshape(C, C)
    g1c = g1[:, None]
    wqkv = np.concatenate([g1c * Wqf * CINV, g1c * Wkf, g1c * Wvf],
                          axis=1).astype(f16)
    bq = (be1 @ Wqf) * CINV
    bk = be1 @ Wkf
    bv = be1 @ Wvf
    bqk = np.stack([bq[:128], bq[128:], bk[:128], bk[128:]], axis=1).astype(f32)
    w1 = (g2[:, None] * W1).astype(f16)
    b1e = (b1 + be2 @ W1).astype(f32)
    b1m = np.ascontiguousarray(b1e.reshape(8, 128).T)  # [128, 8]
    brows = np.zeros((128, C), dtype=f16)  # rows 0/32/64 = bv/bo/b2
    brows[0] = bv.astype(f16)
    brows[32] = bo.astype(f16)
    brows[64] = b2.astype(f16)
    # causal mask [128, 4*128] fp16 (tiled over 4 heads):
    # multiplicative 0/1 by default, additive 0/NEG when MASK_MM
    if MASK_MM:
        m = np.full((128, 128), NEG, dtype=f32)
        for j in range(128 // T):
            m[j * T:(j + 1) * T, j * T:(j + 1) * T] = np.triu(
                np.full((T, T), NEG, dtype=f32), 1)
    else:
        m = np.zeros((128, 128), dtype=f32)
        for j in range(128 // T):
            m[j * T:(j + 1) * T, j * T:(j + 1) * T] = np.tril(
                np.ones((T, T), dtype=f32))
    mask4 = np.tile(m, (1, H)).astype(f16)
    return dict(wqkv=wqkv, bqk=bqk, wo=Wo.astype(f16), w1=w1, b1m=b1m,
                w2=W2.astype(f16), brows=brows, mask4=mask4,
                ident=np.eye(128, dtype=f16))


_CACHE = {}


def kernel(x, Wq, Wk, Wv, Wo, bo, W1, b1, W2, b2, g1, be1, g2, be2):
    x = np.asarray(x, dtype=np.float32)
    wts = _prep_weights(np.asarray(Wq), np.asarray(Wk), np.asarray(Wv),
                        np.asarray(Wo), np.asarray(bo), np.asarray(W1),
                        np.asarray(b1), np.asarray(W2), np.asarray(b2),
                        np.asarray(g1), np.asarray(be1), np.asarray(g2),
                        np.asarray(be2))
    use_bias = bool(np.any(wts["brows"])) or \
        _os.environ.get("FORCE_BIAS", "0") == "1"
    key = ("nc", use_bias)
    if key not in _CACHE:
        _CACHE[key] = build_kernel(use_bias_mms=use_bias)
    nc = _CACHE[key]
    xs = x.reshape(NCORES, NTOK, C)
    in_maps = [dict(x=np.ascontiguousarray(xs[i]), **wts) for i in range(NCORES)]
    kres = run_bass_kernel_spmd(
        nc, in_maps, list(range(NCORES)),
        trace=_os.environ.get("KERNEL_TRACE", "0") == "1",
        tmpdir=_os.environ.get("KERNEL_TRACE_DIR") or None)
    _CACHE["last"] = kres
    res = kres.results
    out = np.stack([res[i]["out"] for i in range(NCORES)], axis=0)
    return out.reshape(B, T, C).astype(np.float32)


if __name__ == "__main__":
    nc = build_kernel()
    print("kernel traced OK")
